# revision 16
# baseline (speedup 1.0000x reference)
"""Multi-head causal attention (B=4, S=2048, D=1024, H=16) on 8 TRN2 cores.

Sharding: tensor-parallel over heads (2 heads/core), proj_out row-parallel
with the cross-core reduction done host-side during unsharding.

Per-core kernel layout (all contractions on the SBUF partition axis):
  xT      (1024 d, 8192 tok)   host-pretransposed activations (shared input)
  qT/kT   (128 e2, 2048 s)     per batch; e2 = 2 heads x 64
  scoresT (128 sk, 512 sq)     kv-major scores -> exp -> PV matmul directly
  denom   ones-matmul broadcast of the per-column sums of exp(scores)
  ctxT    (128 e2, 512 sq)     normalized, fed straight into row-parallel Wo
  outp    (1024 o, 8192 tok)   per-core partial; host sums over cores
"""

import sys

if "/opt/trn_rl_repo" not in sys.path:
    sys.path.insert(0, "/opt/trn_rl_repo")

from contextlib import ExitStack

import numpy as np

import concourse.bass as bass
import concourse.bacc as bacc
import concourse.mybir as mybir
import concourse.tile as tile
from concourse.bass_utils import run_bass_kernel_spmd
from concourse.masks import make_identity

B, S, D, H, E = 4, 2048, 1024, 16, 64
NCORES = 8
HL = H // NCORES          # heads per core = 2
EL = HL * E               # local feature width = 128
SQ = 512                  # query chunk (matmul moving dim)
NQ = S // SQ              # 4
KT = 128                  # kv tile (contraction tile)
DT = 128                  # d-model contraction tile
ND = D // DT              # 8
F32 = mybir.dt.float32
BF16 = mybir.dt.bfloat16
EXP = mybir.ActivationFunctionType.Exp

LAST_RESULTS = None


def build():
    nc = bacc.Bacc()
    xT = nc.declare_dram_parameter("xT", [D, B * S], BF16, isOutput=False)
    wqkv = nc.declare_dram_parameter("wqkv", [ND, DT, 3 * EL], BF16, isOutput=False)
    wo = nc.declare_dram_parameter("wo", [EL, D], BF16, isOutput=False)
    masks = nc.declare_dram_parameter("masks", [KT, NQ, SQ], BF16, isOutput=False)
    outp = nc.declare_dram_parameter("outp", [D, B * S], F32, isOutput=True)

    with tile.TileContext(nc) as tc, ExitStack() as ctx:
        consts = ctx.enter_context(tc.tile_pool(name="consts", bufs=1))
        xt_pool = ctx.enter_context(tc.tile_pool(name="xt", bufs=2))
        qk_pool = ctx.enter_context(tc.tile_pool(name="qk", bufs=2))
        ex_pool = ctx.enter_context(tc.tile_pool(name="ex", bufs=2))
        misc_pool = ctx.enter_context(tc.tile_pool(name="misc", bufs=2))
        out_pool = ctx.enter_context(tc.tile_pool(name="outsb", bufs=2))
        mm_psum = ctx.enter_context(tc.tile_pool(name="mmps", bufs=2, space="PSUM"))
        sc_psum = ctx.enter_context(tc.tile_pool(name="scps", bufs=1, space="PSUM"))
        acc_psum = ctx.enter_context(tc.tile_pool(name="accps", bufs=2, space="PSUM"))

        wqkv_sb = consts.tile([DT, ND, 3 * EL], BF16)
        nc.sync.dma_start(wqkv_sb[:], wqkv[:, :, :].rearrange("t p c -> p t c"))
        wo_sb = consts.tile([EL, D], BF16)
        nc.sync.dma_start(wo_sb[:], wo[:])
        masks_sb = consts.tile([KT, NQ, SQ], BF16)
        nc.sync.dma_start(masks_sb[:], masks[:])
        ones_sb = consts.tile([DT, DT], BF16)
        nc.vector.memset(ones_sb[:], 1.0)
        ident = consts.tile([DT, DT], BF16)
        make_identity(nc, ident[:])

        for b in range(B):
            # ---- q/k/v projections (transposed layout), x^T staged per chunk ----
            qT = qk_pool.tile([EL, S], BF16, name=f"qT_{b}", tag="qT")
            kT = qk_pool.tile([EL, S], BF16, name=f"kT_{b}", tag="kT")
            vT = qk_pool.tile([EL, S], BF16, name=f"vT_{b}", tag="vT", bufs=1)
            for c in range(NQ):
                xt8 = xt_pool.tile([DT, ND, SQ], BF16, name=f"xt_{b}_{c}", tag="xt")
                nc.sync.dma_start(
                    xt8[:],
                    xT[:, b * S + c * SQ: b * S + (c + 1) * SQ].rearrange(
                        "(t p) n -> p t n", p=DT
                    ),
                )
                for dest, col0 in ((qT, 0), (kT, EL), (vT, 2 * EL)):
                    ps = mm_psum.tile([EL, SQ], F32, name=f"qkv_ps_{b}_{c}_{col0}", tag="mm")
                    for t in range(ND):
                        nc.tensor.matmul(
                            ps[:],
                            wqkv_sb[:, t, col0:col0 + EL],
                            xt8[:, t, :],
                            start=(t == 0),
                            stop=(t == ND - 1),
                        )
                    nc.vector.tensor_copy(dest[:, c * SQ:(c + 1) * SQ], ps[:])

            # ---- v: transpose to (sk, e2) tiles ----
            v_sb = qk_pool.tile([KT, S // KT, EL], BF16, name=f"v_{b}", tag="v")
            for j in range(S // KT):
                vt_ps = mm_psum.tile([KT, KT], BF16, name=f"vt_ps_{b}_{j}", tag="mm")
                nc.tensor.transpose(vt_ps[:], vT[:, j * KT:(j + 1) * KT], ident[:])
                nc.scalar.copy(v_sb[:, j, :], vt_ps[:])

            # ---- attention per query chunk ----
            for c in range(NQ):
                J = (c + 1) * (SQ // KT)  # causal kv tiles for this chunk
                ctx_ps = acc_psum.tile([2 * E, SQ], F32, name=f"ctx_{b}_{c}", tag="ctx")
                P2 = misc_pool.tile([KT, 2, SQ], F32, name=f"P2_{b}_{c}", tag="P2")
                for jj in range(0, J, 2):
                    sc = sc_psum.tile([KT, 4, SQ], F32, name=f"sc_{b}_{c}_{jj}", tag="sc")
                    ex = ex_pool.tile([KT, 4, SQ], BF16, name=f"ex_{b}_{c}_{jj}", tag="ex")
                    for dj in range(2):
                        j = jj + dj
                        for h in range(HL):
                            nc.tensor.matmul(
                                sc[:, 2 * dj + h, :],
                                kT[h * E:(h + 1) * E, j * KT:(j + 1) * KT],
                                qT[h * E:(h + 1) * E, c * SQ:(c + 1) * SQ],
                                start=True,
                                stop=True,
                            )
                    nc.scalar.activation(ex[:, :, :], sc[:, :, :], EXP, scale=0.125)
                    for dj in range(2):
                        j = jj + dj
                        rdiag = j - (c * (SQ // KT))
                        if rdiag >= 0:
                            for h in range(HL):
                                nc.gpsimd.tensor_mul(
                                    ex[:, 2 * dj + h, :],
                                    ex[:, 2 * dj + h, :],
                                    masks_sb[:, rdiag, :],
                                )
                        if j == 0:
                            nc.vector.tensor_copy(P2[:, :, :], ex[:, 0:2, :])
                        else:
                            nc.vector.tensor_add(
                                P2[:, :, :], P2[:, :, :], ex[:, 2 * dj:2 * dj + 2, :]
                            )
                        for h in range(HL):
                            nc.tensor.matmul(
                                ctx_ps[h * E:(h + 1) * E, :],
                                v_sb[:, j, h * E:(h + 1) * E],
                                ex[:, 2 * dj + h, :],
                                start=(j == 0),
                                stop=(j == J - 1),
                                skip_group_check=True,
                            )

                # ---- softmax denominator: broadcast-sum then reciprocal ----
                P2b = misc_pool.tile([KT, 2, SQ], BF16, name=f"P2b_{b}_{c}", tag="P2b")
                nc.vector.tensor_copy(P2b[:], P2[:])
                denb = mm_psum.tile([KT, SQ], F32, name=f"den_{b}_{c}", tag="mm")
                for h in range(HL):
                    nc.tensor.matmul(
                        denb[h * E:(h + 1) * E, :],
                        ones_sb[:, h * E:(h + 1) * E],
                        P2b[:, h, :],
                        start=True,
                        stop=True,
                        skip_group_check=True,
                    )
                recb = misc_pool.tile([KT, SQ], F32, name=f"rec_{b}_{c}", tag="recb")
                nc.vector.reciprocal(recb[:], denb[:])
                ctx_sb = misc_pool.tile([2 * E, SQ], BF16, name=f"ctxsb_{b}_{c}", tag="ctxsb")
                nc.vector.tensor_mul(ctx_sb[:], ctx_ps[:], recb[:])

                # ---- row-parallel output projection (partial) ----
                for o in range(D // DT):
                    ops = mm_psum.tile([DT, SQ], F32, name=f"op_{b}_{c}_{o}", tag="mm")
                    nc.tensor.matmul(
                        ops[:], wo_sb[:, o * DT:(o + 1) * DT], ctx_sb[:],
                        start=True, stop=True,
                    )
                    osb = out_pool.tile([DT, SQ], F32, name=f"osb_{b}_{c}_{o}", tag="osb")
                    if o % 2 == 0:
                        nc.scalar.copy(osb[:], ops[:])
                    else:
                        nc.vector.tensor_copy(osb[:], ops[:])
                    nc.sync.dma_start(
                        outp[o * DT:(o + 1) * DT, b * S + c * SQ: b * S + (c + 1) * SQ],
                        osb[:],
                    )

    nc.finalize()
    return nc


def _host_inputs(x, Wq, Wk, Wv, Wo):
    import ml_dtypes

    bf = ml_dtypes.bfloat16
    xT = np.ascontiguousarray(x.reshape(B * S, D).T).astype(bf)
    p = np.arange(KT)[:, None, None]
    rr = np.arange(NQ)[None, :, None]
    cc = np.arange(SQ)[None, None, :]
    masks = (cc >= KT * rr + p).astype(bf)
    in_maps = []
    for core in range(NCORES):
        hs = slice(core * HL, (core + 1) * HL)
        wq = Wq[hs].reshape(EL, D).T
        wk = Wk[hs].reshape(EL, D).T
        wv = Wv[hs].reshape(EL, D).T
        wqkv = np.ascontiguousarray(
            np.concatenate([wq, wk, wv], axis=1)
        ).astype(bf).reshape(ND, DT, 3 * EL)
        woL = np.ascontiguousarray(
            Wo[:, core * EL:(core + 1) * EL].T
        ).astype(bf)
        in_maps.append({"xT": xT, "wqkv": wqkv, "wo": woL, "masks": masks})
    return in_maps


def kernel(x, Wq, Wk, Wv, Wo):
    global LAST_RESULTS
    x, Wq, Wk, Wv, Wo = (np.asarray(a, dtype=np.float32) for a in (x, Wq, Wk, Wv, Wo))
    nc = build()
    in_maps = _host_inputs(x, Wq, Wk, Wv, Wo)
    import os
    res = run_bass_kernel_spmd(
        nc, in_maps, list(range(NCORES)),
        trace=bool(os.environ.get("BASS_KERNEL_TRACE")),
    )
    LAST_RESULTS = res
    acc = np.zeros((D, B * S), np.float32)
    for rmap in res.results:
        acc += rmap["outp"]
    return np.ascontiguousarray(acc.T).reshape(B, S, D)


if __name__ == "__main__":
    rng = np.random.default_rng(0)
    scale = 1.0 / np.sqrt(D)
    x = rng.standard_normal((B, S, D), dtype=np.float32)
    Wq = rng.standard_normal((H, E, D), dtype=np.float32) * scale
    Wk = rng.standard_normal((H, E, D), dtype=np.float32) * scale
    Wv = rng.standard_normal((H, E, D), dtype=np.float32) * scale
    Wo = rng.standard_normal((D, D), dtype=np.float32) * scale
    out = kernel(x, Wq, Wk, Wv, Wo)
    print(out.shape, out.dtype, float(np.abs(out).max()))


# revision 20
# speedup vs baseline: 1.3464x; 1.3464x over previous
"""Multi-head causal attention (B=4, S=2048, D=1024, H=16) on 8 TRN2 cores.

Sharding: tensor-parallel over heads (2 heads/core), proj_out row-parallel
with the cross-core reduction done host-side during unsharding.

Per-core kernel layout (all contractions on the SBUF partition axis):
  xT      (1024 d, 8192 tok)   host-pretransposed activations (shared input)
  qT/kT   (128 e2, 2048 s)     per batch; e2 = 2 heads x 64
  scoresT (128 sk, 512 sq)     kv-major scores -> exp -> PV matmul directly
  denom   ones-matmul broadcast of the per-column sums of exp(scores)
  ctxT    (128 e2, 512 sq)     normalized, fed straight into row-parallel Wo
  outp    (1024 o, 8192 tok)   per-core partial; host sums over cores
"""

import sys

if "/opt/trn_rl_repo" not in sys.path:
    sys.path.insert(0, "/opt/trn_rl_repo")

from contextlib import ExitStack

import numpy as np

import concourse.bass as bass
import concourse.bacc as bacc
import concourse.mybir as mybir
import concourse.tile as tile
from concourse.bass_utils import run_bass_kernel_spmd
from concourse.masks import make_identity

B, S, D, H, E = 4, 2048, 1024, 16, 64
NCORES = 8
HL = H // NCORES          # heads per core = 2
EL = HL * E               # local feature width = 128
SQ = 512                  # query chunk (matmul moving dim)
NQ = S // SQ              # 4
KT = 128                  # kv tile (contraction tile)
DT = 128                  # d-model contraction tile
ND = D // DT              # 8
F32 = mybir.dt.float32
BF16 = mybir.dt.bfloat16
EXP = mybir.ActivationFunctionType.Exp

LAST_RESULTS = None


def build():
    nc = bacc.Bacc()
    xT = nc.declare_dram_parameter("xT", [D, B * S], BF16, isOutput=False)
    wqkv = nc.declare_dram_parameter("wqkv", [ND, DT, 3 * EL], BF16, isOutput=False)
    wo = nc.declare_dram_parameter("wo", [EL, D], BF16, isOutput=False)
    masks = nc.declare_dram_parameter("masks", [KT, NQ, SQ], BF16, isOutput=False)
    outp = nc.declare_dram_parameter("outp", [D, B * S], F32, isOutput=True)

    with tile.TileContext(nc) as tc, ExitStack() as ctx:
        consts = ctx.enter_context(tc.tile_pool(name="consts", bufs=1))
        xt_pool = ctx.enter_context(tc.tile_pool(name="xt", bufs=2))
        qk_pool = ctx.enter_context(tc.tile_pool(name="qk", bufs=2))
        ex_pool = ctx.enter_context(tc.tile_pool(name="ex", bufs=2))
        misc_pool = ctx.enter_context(tc.tile_pool(name="misc", bufs=2))
        out_pool = ctx.enter_context(tc.tile_pool(name="outsb", bufs=2))
        mm_psum = ctx.enter_context(tc.tile_pool(name="mmps", bufs=2, space="PSUM"))
        sc_psum = ctx.enter_context(tc.tile_pool(name="scps", bufs=1, space="PSUM"))
        acc_psum = ctx.enter_context(tc.tile_pool(name="accps", bufs=1, space="PSUM"))
        den_psum = ctx.enter_context(tc.tile_pool(name="denps", bufs=1, space="PSUM"))

        wqkv_sb = consts.tile([DT, ND, 3 * EL], BF16)
        nc.sync.dma_start(wqkv_sb[:], wqkv[:, :, :].rearrange("t p c -> p t c"))
        wo_sb = consts.tile([EL, D], BF16)
        nc.sync.dma_start(wo_sb[:], wo[:])
        masks_sb = consts.tile([KT, NQ, SQ], BF16)
        nc.sync.dma_start(masks_sb[:], masks[:])
        ones_sb = consts.tile([DT, DT], BF16)
        nc.vector.memset(ones_sb[:], 1.0)
        ident = consts.tile([DT, DT], BF16)
        make_identity(nc, ident[:])

        for b in range(B):
            # ---- q/k/v projections (transposed layout), x^T staged per chunk ----
            qT = qk_pool.tile([EL, S], BF16, name=f"qT_{b}", tag="qT")
            kT = qk_pool.tile([EL, S], BF16, name=f"kT_{b}", tag="kT")
            vT = qk_pool.tile([EL, S], BF16, name=f"vT_{b}", tag="vT", bufs=1)
            for c in range(NQ):
                xt8 = xt_pool.tile([DT, ND, SQ], BF16, name=f"xt_{b}_{c}", tag="xt")
                nc.sync.dma_start(
                    xt8[:],
                    xT[:, b * S + c * SQ: b * S + (c + 1) * SQ].rearrange(
                        "(t p) n -> p t n", p=DT
                    ),
                )
                for dest, col0 in ((qT, 0), (kT, EL), (vT, 2 * EL)):
                    ps = mm_psum.tile([EL, SQ], F32, name=f"qkv_ps_{b}_{c}_{col0}", tag="mm")
                    for t in range(ND):
                        nc.tensor.matmul(
                            ps[:],
                            wqkv_sb[:, t, col0:col0 + EL],
                            xt8[:, t, :],
                            start=(t == 0),
                            stop=(t == ND - 1),
                        )
                    nc.vector.tensor_copy(dest[:, c * SQ:(c + 1) * SQ], ps[:])

            # ---- v: transpose to (sk, e2) tiles ----
            v_sb = qk_pool.tile([KT, S // KT, EL], BF16, name=f"v_{b}", tag="v")
            for j in range(S // KT):
                vt_ps = mm_psum.tile([KT, KT], BF16, name=f"vt_ps_{b}_{j}", tag="mm")
                nc.tensor.transpose(vt_ps[:], vT[:, j * KT:(j + 1) * KT], ident[:])
                nc.vector.tensor_copy(v_sb[:, j, :], vt_ps[:])

            # ---- attention per query chunk ----
            for c in range(NQ):
                J = (c + 1) * (SQ // KT)  # causal kv tiles for this chunk
                ctx_ps = acc_psum.tile([2 * E, SQ], F32, name=f"ctx_{b}_{c}", tag="ctx")
                denb = den_psum.tile([KT, SQ], F32, name=f"den_{b}_{c}", tag="den")
                for jj in range(0, J, 2):
                    sc = sc_psum.tile([KT, 4, SQ], F32, name=f"sc_{b}_{c}_{jj}", tag="sc")
                    ex = ex_pool.tile([KT, 4, SQ], BF16, name=f"ex_{b}_{c}_{jj}", tag="ex")
                    for dj in range(2):
                        j = jj + dj
                        for h in range(HL):
                            nc.tensor.matmul(
                                sc[:, 2 * dj + h, :],
                                kT[h * E:(h + 1) * E, j * KT:(j + 1) * KT],
                                qT[h * E:(h + 1) * E, c * SQ:(c + 1) * SQ],
                                start=True,
                                stop=True,
                            )
                    nc.scalar.activation(ex[:, :, :], sc[:, :, :], EXP, scale=0.125)
                    for dj in range(2):
                        j = jj + dj
                        rdiag = j - (c * (SQ // KT))
                        if rdiag >= 0:
                            for h in range(HL):
                                nc.vector.tensor_mul(
                                    ex[:, 2 * dj + h, :],
                                    ex[:, 2 * dj + h, :],
                                    masks_sb[:, rdiag, :],
                                )
                        for h in range(HL):
                            nc.tensor.matmul(
                                ctx_ps[h * E:(h + 1) * E, :],
                                v_sb[:, j, h * E:(h + 1) * E],
                                ex[:, 2 * dj + h, :],
                                start=(j == 0),
                                stop=(j == J - 1),
                                skip_group_check=True,
                            )
                        # denominator rides PE: ones.T @ ex accumulates the
                        # per-column sums, already broadcast over partitions
                        for h in range(HL):
                            nc.tensor.matmul(
                                denb[h * E:(h + 1) * E, :],
                                ones_sb[:, h * E:(h + 1) * E],
                                ex[:, 2 * dj + h, :],
                                start=(j == 0),
                                stop=(j == J - 1),
                                skip_group_check=True,
                            )

                recb = misc_pool.tile([KT, SQ], F32, name=f"rec_{b}_{c}", tag="recb")
                nc.vector.reciprocal(recb[:], denb[:])
                ctx_sb = misc_pool.tile([2 * E, SQ], BF16, name=f"ctxsb_{b}_{c}", tag="ctxsb")
                nc.vector.tensor_mul(ctx_sb[:], ctx_ps[:], recb[:])

                # ---- row-parallel output projection (partial) ----
                for o in range(D // DT):
                    ops = mm_psum.tile([DT, SQ], F32, name=f"op_{b}_{c}_{o}", tag="mm")
                    nc.tensor.matmul(
                        ops[:], wo_sb[:, o * DT:(o + 1) * DT], ctx_sb[:],
                        start=True, stop=True,
                    )
                    osb = out_pool.tile([DT, SQ], F32, name=f"osb_{b}_{c}_{o}", tag="osb")
                    nc.vector.tensor_copy(osb[:], ops[:])
                    nc.sync.dma_start(
                        outp[o * DT:(o + 1) * DT, b * S + c * SQ: b * S + (c + 1) * SQ],
                        osb[:],
                    )

    nc.finalize()
    return nc


def _host_inputs(x, Wq, Wk, Wv, Wo):
    import ml_dtypes

    bf = ml_dtypes.bfloat16
    xT = np.ascontiguousarray(x.reshape(B * S, D).T).astype(bf)
    p = np.arange(KT)[:, None, None]
    rr = np.arange(NQ)[None, :, None]
    cc = np.arange(SQ)[None, None, :]
    masks = (cc >= KT * rr + p).astype(bf)
    in_maps = []
    for core in range(NCORES):
        hs = slice(core * HL, (core + 1) * HL)
        wq = Wq[hs].reshape(EL, D).T
        wk = Wk[hs].reshape(EL, D).T
        wv = Wv[hs].reshape(EL, D).T
        wqkv = np.ascontiguousarray(
            np.concatenate([wq, wk, wv], axis=1)
        ).astype(bf).reshape(ND, DT, 3 * EL)
        woL = np.ascontiguousarray(
            Wo[:, core * EL:(core + 1) * EL].T
        ).astype(bf)
        in_maps.append({"xT": xT, "wqkv": wqkv, "wo": woL, "masks": masks})
    return in_maps


def kernel(x, Wq, Wk, Wv, Wo):
    global LAST_RESULTS
    x, Wq, Wk, Wv, Wo = (np.asarray(a, dtype=np.float32) for a in (x, Wq, Wk, Wv, Wo))
    nc = build()
    in_maps = _host_inputs(x, Wq, Wk, Wv, Wo)
    import os
    res = run_bass_kernel_spmd(
        nc, in_maps, list(range(NCORES)),
        trace=bool(os.environ.get("BASS_KERNEL_TRACE")),
    )
    LAST_RESULTS = res
    acc = np.zeros((D, B * S), np.float32)
    for rmap in res.results:
        acc += rmap["outp"]
    return np.ascontiguousarray(acc.T).reshape(B, S, D)


if __name__ == "__main__":
    rng = np.random.default_rng(0)
    scale = 1.0 / np.sqrt(D)
    x = rng.standard_normal((B, S, D), dtype=np.float32)
    Wq = rng.standard_normal((H, E, D), dtype=np.float32) * scale
    Wk = rng.standard_normal((H, E, D), dtype=np.float32) * scale
    Wv = rng.standard_normal((H, E, D), dtype=np.float32) * scale
    Wo = rng.standard_normal((D, D), dtype=np.float32) * scale
    out = kernel(x, Wq, Wk, Wv, Wo)
    print(out.shape, out.dtype, float(np.abs(out).max()))


# revision 23
# speedup vs baseline: 1.7149x; 1.2737x over previous
"""Multi-head causal attention (B=4, S=2048, D=1024, H=16) on 8 TRN2 cores.

Sharding: tensor-parallel over heads (2 heads/core), proj_out row-parallel
with the cross-core reduction done host-side during unsharding.

Per-core kernel layout (all contractions on the SBUF partition axis):
  xT      (1024 d, 8192 tok)   host-pretransposed activations (shared input)
  qT/kT   (128 e2, 2048 s)     per batch; e2 = 2 heads x 64
  scoresT (128 sk, 512 sq)     kv-major scores -> exp -> PV matmul directly
  denom   ones-matmul broadcast of the per-column sums of exp(scores)
  ctxT    (128 e2, 512 sq)     normalized, fed straight into row-parallel Wo
  outp    (1024 o, 8192 tok)   per-core partial; host sums over cores
"""

import sys

if "/opt/trn_rl_repo" not in sys.path:
    sys.path.insert(0, "/opt/trn_rl_repo")

from contextlib import ExitStack

import numpy as np

import concourse.bass as bass
import concourse.bacc as bacc
import concourse.mybir as mybir
import concourse.tile as tile
from concourse.bass_utils import run_bass_kernel_spmd
from concourse.masks import make_identity

B, S, D, H, E = 4, 2048, 1024, 16, 64
NCORES = 8
HL = H // NCORES          # heads per core = 2
EL = HL * E               # local feature width = 128
SQ = 512                  # query chunk (matmul moving dim)
NQ = S // SQ              # 4
KT = 128                  # kv tile (contraction tile)
DT = 128                  # d-model contraction tile
ND = D // DT              # 8
F32 = mybir.dt.float32
BF16 = mybir.dt.bfloat16
EXP = mybir.ActivationFunctionType.Exp

LAST_RESULTS = None


def build():
    nc = bacc.Bacc()
    xT = nc.declare_dram_parameter("xT", [D, B * S], BF16, isOutput=False)
    wqkv = nc.declare_dram_parameter("wqkv", [ND, DT, 3 * EL], BF16, isOutput=False)
    wo = nc.declare_dram_parameter("wo", [EL, D], BF16, isOutput=False)
    masks = nc.declare_dram_parameter("masks", [KT, NQ, SQ], BF16, isOutput=False)
    outp = nc.declare_dram_parameter("outp", [D, B * S], F32, isOutput=True)

    with tile.TileContext(nc) as tc, ExitStack() as ctx:
        consts = ctx.enter_context(tc.tile_pool(name="consts", bufs=1))
        xt_pool = ctx.enter_context(tc.tile_pool(name="xt", bufs=2))
        qk_pool = ctx.enter_context(tc.tile_pool(name="qk", bufs=2))
        ex_pool = ctx.enter_context(tc.tile_pool(name="ex", bufs=4))
        misc_pool = ctx.enter_context(tc.tile_pool(name="misc", bufs=2))
        out_pool = ctx.enter_context(tc.tile_pool(name="outsb", bufs=2))
        mm_psum = ctx.enter_context(tc.tile_pool(name="mmps", bufs=2, space="PSUM"))
        sc_psum = ctx.enter_context(tc.tile_pool(name="scps", bufs=2, space="PSUM"))
        acc_psum = ctx.enter_context(tc.tile_pool(name="accps", bufs=1, space="PSUM"))
        den_psum = ctx.enter_context(tc.tile_pool(name="denps", bufs=1, space="PSUM"))

        wqkv_sb = consts.tile([DT, ND, 3 * EL], BF16)
        nc.sync.dma_start(wqkv_sb[:], wqkv[:, :, :].rearrange("t p c -> p t c"))
        wo_sb = consts.tile([EL, D], BF16)
        nc.sync.dma_start(wo_sb[:], wo[:])
        masks_sb = consts.tile([KT, NQ, SQ], BF16)
        nc.sync.dma_start(masks_sb[:], masks[:])
        ones_sb = consts.tile([DT, DT], BF16)
        nc.vector.memset(ones_sb[:], 1.0)
        ident = consts.tile([DT, DT], BF16)
        make_identity(nc, ident[:])

        for b in range(B):
            # ---- q/k/v projections (transposed layout), x^T staged per chunk ----
            qT = qk_pool.tile([EL, S], BF16, name=f"qT_{b}", tag="qT")
            kT = qk_pool.tile([EL, S], BF16, name=f"kT_{b}", tag="kT")
            vT = qk_pool.tile([EL, S], BF16, name=f"vT_{b}", tag="vT", bufs=1)
            for c in range(NQ):
                xt8 = xt_pool.tile([DT, ND, SQ], BF16, name=f"xt_{b}_{c}", tag="xt")
                nc.sync.dma_start(
                    xt8[:],
                    xT[:, b * S + c * SQ: b * S + (c + 1) * SQ].rearrange(
                        "(t p) n -> p t n", p=DT
                    ),
                )
                for dest, col0 in ((qT, 0), (kT, EL), (vT, 2 * EL)):
                    ps = mm_psum.tile([EL, SQ], F32, name=f"qkv_ps_{b}_{c}_{col0}", tag="mm")
                    for t in range(ND):
                        nc.tensor.matmul(
                            ps[:],
                            wqkv_sb[:, t, col0:col0 + EL],
                            xt8[:, t, :],
                            start=(t == 0),
                            stop=(t == ND - 1),
                        )
                    nc.vector.tensor_copy(dest[:, c * SQ:(c + 1) * SQ], ps[:])

            # ---- v: transpose to (sk, e2) tiles ----
            v_sb = qk_pool.tile([KT, S // KT, EL], BF16, name=f"v_{b}", tag="v")
            for j in range(S // KT):
                vt_ps = mm_psum.tile([KT, KT], BF16, name=f"vt_ps_{b}_{j}", tag="mm")
                nc.tensor.transpose(vt_ps[:], vT[:, j * KT:(j + 1) * KT], ident[:])
                nc.vector.tensor_copy(v_sb[:, j, :], vt_ps[:])

            # ---- attention per query chunk ----
            for c in range(NQ):
                J = (c + 1) * (SQ // KT)  # causal kv tiles for this chunk
                ctx_ps = acc_psum.tile([2 * E, SQ], F32, name=f"ctx_{b}_{c}", tag="ctx")
                denb = den_psum.tile([KT, SQ], F32, name=f"den_{b}_{c}", tag="den")
                for j in range(J):
                    rdiag = j - (c * (SQ // KT))
                    # columns [0, cut) of this q-chunk are fully masked for
                    # diagonal kv tiles -- skip them everywhere
                    cut = KT * rdiag if rdiag > 0 else 0
                    n = SQ - cut
                    sc = sc_psum.tile([KT, 2, SQ], F32, name=f"sc_{b}_{c}_{j}", tag="sc")
                    ex = ex_pool.tile([KT, 2, SQ], BF16, name=f"ex_{b}_{c}_{j}", tag="ex")
                    for h in range(HL):
                        nc.tensor.matmul(
                            sc[:, h, 0:n],
                            kT[h * E:(h + 1) * E, j * KT:(j + 1) * KT],
                            qT[h * E:(h + 1) * E, c * SQ + cut:(c + 1) * SQ],
                            start=True,
                            stop=True,
                        )
                    if cut:
                        nc.gpsimd.memset(ex[:, :, 0:cut], 0.0)
                    nc.scalar.activation(
                        ex[:, :, cut:SQ], sc[:, :, 0:n], EXP, scale=0.125
                    )
                    if rdiag >= 0:
                        for h in range(HL):
                            nc.vector.tensor_mul(
                                ex[:, h, cut:SQ],
                                ex[:, h, cut:SQ],
                                masks_sb[:, rdiag, cut:SQ],
                            )
                    for h in range(HL):
                        nc.tensor.matmul(
                            ctx_ps[h * E:(h + 1) * E, cut:SQ],
                            v_sb[:, j, h * E:(h + 1) * E],
                            ex[:, h, cut:SQ],
                            start=(j == 0),
                            stop=(j == J - 1),
                            skip_group_check=True,
                        )
                    # denominator rides PE: ones.T @ ex accumulates the
                    # per-column sums, already broadcast over partitions
                    for h in range(HL):
                        nc.tensor.matmul(
                            denb[h * E:(h + 1) * E, cut:SQ],
                            ones_sb[:, h * E:(h + 1) * E],
                            ex[:, h, cut:SQ],
                            start=(j == 0),
                            stop=(j == J - 1),
                            skip_group_check=True,
                        )

                recb = misc_pool.tile([KT, SQ], F32, name=f"rec_{b}_{c}", tag="recb")
                nc.vector.reciprocal(recb[:], denb[:])
                ctx_sb = misc_pool.tile([2 * E, SQ], BF16, name=f"ctxsb_{b}_{c}", tag="ctxsb")
                nc.vector.tensor_mul(ctx_sb[:], ctx_ps[:], recb[:])

                # ---- row-parallel output projection (partial) ----
                for o in range(D // DT):
                    ops = mm_psum.tile([DT, SQ], F32, name=f"op_{b}_{c}_{o}", tag="mm")
                    nc.tensor.matmul(
                        ops[:], wo_sb[:, o * DT:(o + 1) * DT], ctx_sb[:],
                        start=True, stop=True,
                    )
                    osb = out_pool.tile([DT, SQ], F32, name=f"osb_{b}_{c}_{o}", tag="osb")
                    nc.vector.tensor_copy(osb[:], ops[:])
                    nc.sync.dma_start(
                        outp[o * DT:(o + 1) * DT, b * S + c * SQ: b * S + (c + 1) * SQ],
                        osb[:],
                    )

    nc.finalize()
    return nc


def _host_inputs(x, Wq, Wk, Wv, Wo):
    import ml_dtypes

    bf = ml_dtypes.bfloat16
    xT = np.ascontiguousarray(x.reshape(B * S, D).T).astype(bf)
    p = np.arange(KT)[:, None, None]
    rr = np.arange(NQ)[None, :, None]
    cc = np.arange(SQ)[None, None, :]
    masks = (cc >= KT * rr + p).astype(bf)
    in_maps = []
    for core in range(NCORES):
        hs = slice(core * HL, (core + 1) * HL)
        wq = Wq[hs].reshape(EL, D).T
        wk = Wk[hs].reshape(EL, D).T
        wv = Wv[hs].reshape(EL, D).T
        wqkv = np.ascontiguousarray(
            np.concatenate([wq, wk, wv], axis=1)
        ).astype(bf).reshape(ND, DT, 3 * EL)
        woL = np.ascontiguousarray(
            Wo[:, core * EL:(core + 1) * EL].T
        ).astype(bf)
        in_maps.append({"xT": xT, "wqkv": wqkv, "wo": woL, "masks": masks})
    return in_maps


def kernel(x, Wq, Wk, Wv, Wo):
    global LAST_RESULTS
    x, Wq, Wk, Wv, Wo = (np.asarray(a, dtype=np.float32) for a in (x, Wq, Wk, Wv, Wo))
    nc = build()
    in_maps = _host_inputs(x, Wq, Wk, Wv, Wo)
    import os
    res = run_bass_kernel_spmd(
        nc, in_maps, list(range(NCORES)),
        trace=bool(os.environ.get("BASS_KERNEL_TRACE")),
    )
    LAST_RESULTS = res
    acc = np.zeros((D, B * S), np.float32)
    for rmap in res.results:
        acc += rmap["outp"]
    return np.ascontiguousarray(acc.T).reshape(B, S, D)


if __name__ == "__main__":
    rng = np.random.default_rng(0)
    scale = 1.0 / np.sqrt(D)
    x = rng.standard_normal((B, S, D), dtype=np.float32)
    Wq = rng.standard_normal((H, E, D), dtype=np.float32) * scale
    Wk = rng.standard_normal((H, E, D), dtype=np.float32) * scale
    Wv = rng.standard_normal((H, E, D), dtype=np.float32) * scale
    Wo = rng.standard_normal((D, D), dtype=np.float32) * scale
    out = kernel(x, Wq, Wk, Wv, Wo)
    print(out.shape, out.dtype, float(np.abs(out).max()))
